# revision 35
# baseline (speedup 1.0000x reference)
"""Fused multi-head attention (QKV proj + RoPE + causal softmax + out proj)
for Trainium2, sharded over 8 NeuronCores.

Sharding: data-parallel over batch (B=2) x tensor-parallel over heads
(16 heads -> 4 per core).  Each core computes, for its (batch, head-group):
  qT/kT = wq/wk^T-projections in [d, s] layout (CDT matmuls, fp32 PSUM)
  RoPE applied on-chip (DVE pair-swap via stream_shuffle + mul/add, read
  directly from PSUM)
  scoresT[kp, q] = krot^T.T @ qrot (one K=128 matmul per tile)
  causal masking on diagonal tiles via GPSIMD affine_select zeroing of the
  exp tile (no PE mask matmuls)
  expT = exp(scale * scoresT) on ACT
  PV with a ones-augmented V column => unnormalized out + softmax denominator
  normalize (DVE reciprocal + tensor_scalar straight from PSUM), PE-transpose
  to attnT[d, s], drained by ACT
  partial output y_g = attnT.T @ wo_rows (bf16, summed over head-groups on
  host)

PSUM discipline: three fixed pools (4+2+2 banks).  Projections alternate
even/odd groups between {a} and {b,c} so accumulation of block g+1 overlaps
the RoPE/copy drain of block g.  Attention: scores double-buffer in b, PV
accumulators in a, transposes + out-proj in c.
"""

import math

import numpy as np

import concourse.bacc as bacc
import concourse.mybir as mybir
from concourse import tile
from concourse.bass_utils import run_bass_kernel_spmd

B, S, D, H = 2, 2048, 2048, 16
NCORES = 8
HG = 4  # heads per core
HD = D // H  # 128
DG = HG * HD  # 512 = per-core slice of D
P = 128
NKC = D // P  # 16 contraction chunks
SBLK = 512  # s-block width in projection passes
NSB = S // SBLK
NST = S // P  # 16 s-tiles of 128
QB = 512  # q-block width in attention
NQB = S // QB
NQT = QB // P  # q-subtiles per block
EB = 512  # e-block width in out-projection
NEB = D // EB

F32 = mybir.dt.float32
EXP = mybir.ActivationFunctionType.Exp
SCALE = 1.0 / math.sqrt(HD)
SWAP32 = [i ^ 1 for i in range(32)]
NEG = -1.0e9

COMPUTE_DTYPE = "bfloat16"


def build_program(variant: str, dump: bool = False, cdt_name: str | None = None):
    """variant: 'causal' | 'none' | 'general'"""
    CDT = getattr(mybir.dt, cdt_name or COMPUTE_DTYPE)
    nc = bacc.Bacc("TRN2", target_bir_lowering=False, debug=False)
    xT = nc.dram_tensor("xT", [D, S], CDT, kind="ExternalInput")
    wq = nc.dram_tensor("wq", [D, DG], CDT, kind="ExternalInput")
    wk = nc.dram_tensor("wk", [D, DG], CDT, kind="ExternalInput")
    wv = nc.dram_tensor("wv", [D, DG], CDT, kind="ExternalInput")
    wo = nc.dram_tensor("wo", [DG, D], CDT, kind="ExternalInput")
    cosT = nc.dram_tensor("cosT", [HD, S], CDT, kind="ExternalInput")
    sinT = nc.dram_tensor("sinT", [HD, S], CDT, kind="ExternalInput")
    ident = nc.dram_tensor("ident", [P, P], CDT, kind="ExternalInput")
    maskT = None
    if variant == "general":
        # mask.T pre-scaled by sqrt(HD) on host so exp's scale recovers it
        maskT = nc.dram_tensor("maskT", [S, S], CDT, kind="ExternalInput")
    y = nc.dram_tensor("y", [S, D], CDT, kind="ExternalOutput")
    d_qrot = d_krot = d_vaug = d_attnT = None
    if dump:
        d_qrot = nc.dram_tensor("d_qrot", [P, HG, S], CDT, kind="ExternalOutput")
        d_krot = nc.dram_tensor("d_krot", [P, HG, S], CDT, kind="ExternalOutput")
        d_vaug = nc.dram_tensor("d_vaug", [P, NST, HG, HD + 2], CDT, kind="ExternalOutput")
        d_attnT = nc.dram_tensor("d_attnT", [P, HG, S], CDT, kind="ExternalOutput")

    with tile.TileContext(nc) as tc:
        with (
            tc.tile_pool(name="const", bufs=1) as constp,
            tc.tile_pool(name="big", bufs=1) as bigp,
            # fixed PSUM partition: 4 + 2 + 2 banks, shared across phases
            # with no pool-scoping barriers
            tc.tile_pool(name="psa", bufs=4, space="PSUM") as ps_a,
            tc.tile_pool(name="psb", bufs=2, space="PSUM") as ps_b,
            tc.tile_pool(name="psc", bufs=2, space="PSUM") as ps_c,
        ):
            qrot = bigp.tile([P, HG, S], CDT, tag="qrot")
            krot = bigp.tile([P, HG, S], CDT, tag="krot")
            vaug = bigp.tile([P, NST, HG, HD + 2], CDT, tag="vaug")
            wo_sb = bigp.tile([P, HG, D], CDT, tag="wo")
            w_all = bigp.tile([P, 3, NKC, DG], CDT, tag="wall")
            tcos = constp.tile([HD, S], CDT, tag="tcos")
            tsin = constp.tile([HD, S], CDT, tag="tsin")
            tid = constp.tile([P, P], CDT, tag="tid")

            def proj_psum(g, width, nun):
                # even groups use the 4-slot pool; odd groups the 2+2 pools,
                # so accumulation and drain of adjacent groups overlap
                if g % 2 == 0:
                    return [
                        ps_a.tile([P, width], F32, tag="pa", name=f"pa{g}_{u}")
                        for u in range(nun)
                    ]
                return [
                    ps_b.tile([P, width], F32, tag="pb", name=f"pb{g}_{u}")
                    for u in range(2)
                ] + [
                    ps_c.tile([P, width], F32, tag="pc", name=f"pc{g}_{u}")
                    for u in range(nun - 2)
                ]

            # ---------------- projections + RoPE ----------------
            # weights are resident in SBUF (loaded once); each x column-block
            # is loaded once and consumed by all three projections
            with (
                tc.tile_pool(name="xpool", bufs=2) as xpool,
                tc.tile_pool(name="rope", bufs=3) as ropep,
            ):
                # sb0's x (scalar queue) and wq (sync queue) stream per-kc so
                # chunk arrival matches the matmul consumption order; wk/wv
                # follow behind on the same queues
                xb0 = xpool.tile([P, NKC, SBLK], CDT, tag="xb", name="xb0")
                for kc in range(NKC):
                    nc.scalar.dma_start(
                        xb0[:, kc, :],
                        xT[kc * P : (kc + 1) * P, 0:SBLK],
                    )
                    nc.sync.dma_start(
                        w_all[:, 0, kc, :], wq[kc * P : (kc + 1) * P, :]
                    )

                def w_chunk(pi, wdram, kc4, eng):
                    eng.dma_start(
                        w_all[:, pi, kc4 * 4 : (kc4 + 1) * 4, :],
                        wdram[kc4 * 4 * P : (kc4 + 1) * 4 * P, :].rearrange(
                            "(c p) d -> p c d", p=P
                        ),
                    )

                nc.scalar.dma_start(tcos[:], cosT[:])
                nc.scalar.dma_start(tsin[:], sinT[:])
                for kc4 in range(NKC // 4):
                    w_chunk(1, wk, kc4, nc.sync)
                # wv rides the otherwise-idle gpsimd queue so it does not
                # compete with xb0/cos/sin on scalar
                for kc4 in range(NKC // 4):
                    w_chunk(2, wv, kc4, nc.gpsimd)
                # wo/identity are not needed until the attention phase:
                # keep them out of the contended startup window
                with tc.tile_wait_until(0.06):
                    nc.sync.dma_start(tid[:], ident[:])
                    for dc, eng in zip(
                        range(HG), (nc.scalar, nc.sync, nc.scalar, nc.sync)
                    ):
                        eng.dma_start(
                            wo_sb[:, dc, :], wo[dc * P : (dc + 1) * P, :]
                        )

                def load_xb(sb):
                    t = xpool.tile([P, NKC, SBLK], CDT, tag="xb", name="xb")
                    for kc4 in range(NKC // 4):
                        nc.gpsimd.dma_start(
                            t[:, kc4 * 4 : (kc4 + 1) * 4, :],
                            xT[
                                kc4 * 4 * P : (kc4 + 1) * 4 * P,
                                sb * SBLK : (sb + 1) * SBLK,
                            ].rearrange("(c p) s -> p c s", p=P),
                        )
                    return t
                ones_view = vaug[:, :, :, HD : HD + 2]
                nc.vector.memset(ones_view, 1.0)

                for sb in range(NSB):
                    xb = xb0 if sb == 0 else load_xb(sb)
                    for pi, proj in enumerate(("q", "k", "v")):
                        g = sb * 3 + pi
                        nun = SBLK // P if proj == "v" else HG
                        width = DG if proj == "v" else SBLK
                        pss = proj_psum(g, width, nun)
                        for kc in range(NKC):
                            if proj in ("q", "k"):
                                for dt in range(HG):
                                    nc.tensor.matmul(
                                        pss[dt][:],
                                        w_all[:, pi, kc, dt * HD : (dt + 1) * HD],
                                        xb[:, kc, :],
                                        start=(kc == 0),
                                        stop=(kc == NKC - 1),
                                    )
                            else:
                                for st in range(SBLK // P):
                                    nc.tensor.matmul(
                                        pss[st][:],
                                        xb[:, kc, st * P : (st + 1) * P],
                                        w_all[:, pi, kc, :],
                                        start=(kc == 0),
                                        stop=(kc == NKC - 1),
                                    )
                        if proj in ("q", "k"):
                            dstbuf = qrot if proj == "q" else krot
                            ssl = slice(sb * SBLK, (sb + 1) * SBLK)
                            # ACT drains each PSUM bank (frees it fast); the
                            # RoPE chain runs on DVE from SBUF
                            for dt in range(HG):
                                qsb = ropep.tile([P, SBLK], CDT, tag="qsb", name="qsb")
                                nc.scalar.copy(qsb[:], pss[dt][:])
                                tsw = ropep.tile([P, SBLK], CDT, tag="tsw", name="tsw")
                                nc.vector.stream_shuffle(tsw[:], qsb[:], SWAP32)
                                t1 = ropep.tile([P, SBLK], CDT, tag="t1", name="t1")
                                nc.vector.tensor_mul(t1[:], qsb[:], tcos[:, ssl])
                                t2 = ropep.tile([P, SBLK], CDT, tag="t2", name="t2")
                                nc.vector.tensor_mul(t2[:], tsw[:], tsin[:, ssl])
                                nc.vector.tensor_add(
                                    dstbuf[:, dt, ssl], t1[:], t2[:]
                                )
                        else:
                            for st in range(SBLK // P):
                                st_g = sb * (SBLK // P) + st
                                nc.scalar.copy(
                                    vaug[:, st_g, :, 0:HD],
                                    pss[st][:].rearrange("p (h d) -> p h d", d=HD),
                                )

            if dump:
                nc.sync.dma_start(d_qrot.ap(), qrot[:])
                nc.sync.dma_start(d_krot.ap(), krot[:])
                nc.sync.dma_start(d_vaug.ap(), vaug[:])

            # ---------------- attention + interleaved out projection ----------------
            with (
                tc.tile_pool(name="attn_out", bufs=1) as atp,
                tc.tile_pool(name="mask", bufs=2) as maskp,
                tc.tile_pool(name="expp", bufs=NST + 3) as epool,
                tc.tile_pool(name="small", bufs=4) as smallp,
                tc.tile_pool(name="normp", bufs=3) as npool,
                tc.tile_pool(name="outp", bufs=4) as outp,
            ):
                attnT = atp.tile([P, HG, S], CDT, tag="attnT")

                def nkt_of(qb):
                    return NQT * (qb + 1) if variant == "causal" else NST

                mask_tiles = {}

                def emit_scores_exp(qb, h, kt):
                    """One scores matmul + exp (+ causal diag zeroing)."""
                    j = kt - NQT * qb  # diag index (causal)
                    ps_s = ps_b.tile([P, QB], F32, tag="pb", name="scores")
                    if variant == "causal" and j >= 0:
                        valid = slice(j * P, QB)
                    else:
                        valid = slice(0, QB)
                    nc.tensor.matmul(
                        ps_s[:, valid],
                        krot[:, h, kt * P : (kt + 1) * P],
                        qrot[:, h, qb * QB + valid.start : (qb + 1) * QB],
                        start=True,
                        stop=variant != "general",
                    )
                    if variant == "general":
                        nc.tensor.matmul(
                            ps_s[:],
                            tid[:],
                            mask_tiles[qb][:, kt, :],
                            start=False,
                            stop=True,
                        )
                    texp = epool.tile([P, QB], CDT, tag="exp", name="exp")
                    nc.scalar.activation(
                        texp[:, valid], ps_s[:, valid], EXP, scale=SCALE
                    )
                    if variant == "causal" and j >= 0:
                        # zero exp where k > q inside the diagonal 128x128
                        # block (iota = q_local - k_local >= 0 keeps)
                        nc.gpsimd.affine_select(
                            texp[:, j * P : (j + 1) * P],
                            texp[:, j * P : (j + 1) * P],
                            pattern=[[1, P]],
                            compare_op=mybir.AluOpType.is_ge,
                            fill=0.0,
                            base=0,
                            channel_multiplier=-1,
                        )
                    return texp

                # out-projection is emitted as fine-grained "quanta" (one
                # dc-matmul each) interleaved through the kt loops so the PE
                # always has exp-independent work while ACT computes exps
                out_q = []
                out_open = {}

                def enqueue_st(st):
                    out_q.extend(
                        (st, eb, dc) for eb in range(NEB) for dc in range(HG)
                    )

                def emit_quantum():
                    if not out_q:
                        return False
                    st, eb, dc = out_q.pop(0)
                    if dc == 0:
                        out_open["ps"] = ps_c.tile(
                            [P, EB], F32, tag="pc", name=f"o{st}_{eb}"
                        )
                    ps_o = out_open["ps"]
                    nc.tensor.matmul(
                        ps_o[:],
                        attnT[:, dc, st * P : (st + 1) * P],
                        wo_sb[:, dc, eb * EB : (eb + 1) * EB],
                        start=(dc == 0),
                        stop=(dc == HG - 1),
                    )
                    if dc == HG - 1:
                        out_t = outp.tile(
                            [P, EB], CDT, tag="outsb", name="outsb"
                        )
                        nc.vector.tensor_copy(out_t[:], ps_o[:])
                        nc.sync.dma_start(
                            y[st * P : (st + 1) * P, eb * EB : (eb + 1) * EB],
                            out_t[:],
                        )
                    return True

                seq = [(qb, h) for qb in range(NQB) for h in range(HG)]
                pre = {}
                for i, (qb, h) in enumerate(seq):
                    nkt = nkt_of(qb)
                    if variant == "general" and h == 0:
                        mt = maskp.tile([P, NST, QB], CDT, tag="mt", name="mt")
                        nc.sync.dma_start(
                            mt[:],
                            maskT[:, qb * QB : (qb + 1) * QB].rearrange(
                                "(kt p) q -> p kt q", p=P
                            ),
                        )
                        mask_tiles[qb] = mt
                    augs = [
                        ps_a.tile([P, HD + 2], F32, tag="pa", name=f"aug{i}")
                        for i in range(4)
                    ]
                    def emit_pvs(kt, texp):
                        for qt in range(NQT):
                            if variant == "causal" and kt > NQT * qb + qt:
                                continue
                            last_kt = (
                                NQT * qb + qt if variant == "causal" else NST - 1
                            )
                            nc.tensor.matmul(
                                augs[qt][:],
                                texp[:, qt * P : (qt + 1) * P],
                                vaug[:, kt, h, :],
                                start=(kt == 0),
                                stop=(kt == last_kt),
                            )
                            if kt == last_kt:
                                # normalize straight from PSUM + transpose;
                                # frees the aug bank fast
                                qt_g = qb * NQT + qt
                                rec = smallp.tile(
                                    [P, 1], F32, tag="rec", name="rec"
                                )
                                nc.vector.reciprocal(
                                    rec[:], augs[qt][:, HD : HD + 1]
                                )
                                attn_n = npool.tile(
                                    [P, HD], CDT, tag="attn_n", name="attn_n"
                                )
                                nc.vector.tensor_scalar_mul(
                                    attn_n[:], augs[qt][:, 0:HD], rec[:]
                                )
                                # cover the DVE normalize latency with an
                                # out-proj quantum before the transpose
                                emit_quantum()
                                ps_t = ps_c.tile([P, P], CDT, tag="pc", name="tr")
                                nc.tensor.transpose(ps_t[:], attn_n[:], tid[:])
                                # drain on DVE: keeps the ACT queue free for
                                # the exp chain at head boundaries
                                nc.vector.tensor_copy(
                                    attnT[:, h, qt_g * P : (qt_g + 1) * P],
                                    ps_t[:],
                                )
                                if h == HG - 1:
                                    # all four heads' columns for this s-tile
                                    # are now in flight: its out-projection
                                    # becomes available filler
                                    enqueue_st(qt_g)

                    # PV accumulation runs one kt behind the scores/exp
                    # pipeline, so each PV's LDWEIGHTS (the exp tile) is
                    # ready early and hides under the previous matmul's fill
                    texps = []
                    for kt in range(nkt):
                        texp = pre.pop(kt, None)
                        if texp is None:
                            texp = emit_scores_exp(qb, h, kt)
                        texps.append(texp)
                        if kt >= 1:
                            emit_pvs(kt - 1, texps[kt - 1])
                        # out-proj quanta keep the PE fed while ACT works
                        # through the exp chain; diagonal k-tiles run only
                        # 1-3 PV matmuls, so they get a second quantum
                        emit_quantum()
                        if variant == "causal" and kt >= NQT * qb:
                            emit_quantum()
                    # pre-issue the next head's first two scores/exps so ACT
                    # computes them under the PV flush + boundary filler;
                    # the first goes out before the flush (its ps_b slot is
                    # long free), the second after
                    pre = {}
                    nxt = seq[i + 1] if i + 1 < len(seq) else None
                    if nxt and variant != "general":
                        pre[0] = emit_scores_exp(nxt[0], nxt[1], 0)
                    emit_pvs(nkt - 1, texps[nkt - 1])
                    if nxt and variant != "general" and nkt_of(nxt[0]) > 1:
                        pre[1] = emit_scores_exp(nxt[0], nxt[1], 1)
                    for _ in range(6):
                        emit_quantum()
                while emit_quantum():
                    pass

                if dump:
                    nc.sync.dma_start(d_attnT.ap(), attnT[:])

    nc.compile()
    return nc


_PROGRAM_CACHE: dict[str, object] = {}
_last_in_maps = None


def _get_program(variant: str):
    key = f"{variant}:{COMPUTE_DTYPE}"
    if key not in _PROGRAM_CACHE:
        _PROGRAM_CACHE[key] = build_program(variant)
    return _PROGRAM_CACHE[key]


def _detect_variant(mask: np.ndarray) -> str:
    if not np.any(mask):
        return "none"
    causal = np.triu(np.full((S, S), NEG, dtype=np.float32), 1)
    if np.array_equal(mask, causal):
        return "causal"
    return "general"


def _np_cdt():
    if COMPUTE_DTYPE == "bfloat16":
        import ml_dtypes

        return ml_dtypes.bfloat16
    return np.float32


def make_in_maps(x, wq, wk, wv, wo, cos, sin, mask, variant):
    npdt = _np_cdt()
    cosT = np.repeat(cos.T, 2, axis=0)  # [HD, S]
    sinT = np.repeat(sin.T, 2, axis=0)
    sinT = sinT.copy()
    sinT[0::2, :] *= -1.0  # row 2i holds -sin, row 2i+1 holds +sin
    shared = {
        "cosT": np.ascontiguousarray(cosT).astype(npdt),
        "sinT": np.ascontiguousarray(sinT).astype(npdt),
        "ident": np.eye(P, dtype=np.float32).astype(npdt),
    }
    if variant == "general":
        shared["maskT"] = np.ascontiguousarray(mask.T * math.sqrt(HD)).astype(npdt)

    xTs = [np.ascontiguousarray(x[b].T).astype(npdt) for b in range(B)]
    in_maps = []
    for core in range(NCORES):
        b, g = divmod(core, NCORES // B)
        sl = slice(g * DG, (g + 1) * DG)
        in_maps.append(
            {
                "xT": xTs[b],
                "wq": np.ascontiguousarray(wq[:, sl]).astype(npdt),
                "wk": np.ascontiguousarray(wk[:, sl]).astype(npdt),
                "wv": np.ascontiguousarray(wv[:, sl]).astype(npdt),
                "wo": np.ascontiguousarray(wo[sl, :]).astype(npdt),
                **shared,
            }
        )
    return in_maps


def kernel(x, wq, wk, wv, wo, cos, sin, mask):
    x = np.asarray(x, dtype=np.float32)
    wq = np.asarray(wq, dtype=np.float32)
    wk = np.asarray(wk, dtype=np.float32)
    wv = np.asarray(wv, dtype=np.float32)
    wo = np.asarray(wo, dtype=np.float32)
    cos = np.asarray(cos, dtype=np.float32)
    sin = np.asarray(sin, dtype=np.float32)
    mask = np.asarray(mask, dtype=np.float32)

    variant = _detect_variant(mask)
    nc = _get_program(variant)
    in_maps = make_in_maps(x, wq, wk, wv, wo, cos, sin, mask, variant)

    global _last_in_maps
    _last_in_maps = in_maps

    res = run_bass_kernel_spmd(nc, in_maps, core_ids=list(range(NCORES)))

    out = np.empty((B, S, D), dtype=np.float32)
    gpb = NCORES // B
    for b in range(B):
        acc = np.zeros((S, D), dtype=np.float64)
        for g in range(gpb):
            acc += res.results[b * gpb + g]["y"].astype(np.float64)
        out[b] = acc.astype(np.float32)
    return out
